# revision 23
# baseline (speedup 1.0000x reference)
"""Trainium2 Bass kernel for dual-modality soft-kNN posterior (retrieval_knn).

Computes, for rgb and flow feature sets:
    dist[q,c] = ||q - p + eps||_2          (torch PairwiseDistance eps)
    post      = exp(-dist) / row_sum
    conf      = row max of post
    ent       = row entropy of post
and select mask (ent_r > ent_f) & (conf_r > conf_f).

Strategy: data-parallel over queries across 8 NeuronCores; prototypes
replicated. Host passes transposed ([D, *]) copies of all feature
matrices so the TensorEngine can contract over D directly.

Math on device (per modality):
    alpha[q] = sum_d q_d^2 + 2*eps*q_d
    beta[c]  = sum_d p_d^2 - 2*eps*p_d      (+ D*eps^2, dropped: < f32 ulp)
    g[q,c]   = sum_d q_d p_d - alpha[q]/2 - beta[c]/2   (K=2 augmented row)
    dist     = sqrt(-2*g)
    u = exp(-dist); S = sum_c u;  post = u/S
    conf = max_c(u)/S;  ent = ln(S) + (sum_c u*dist)/S

Pipelining: classes are processed in two 512-wide halves per modality
(4 phases total). Only one modality's query set plus one prototype
half needs to be resident at a time, which leaves room to prefetch the
next phase's tiles, keeping the TensorEngine dense (and out of the HAM
cold-clock state) across phase boundaries.

ACT table sets: all Sqrt calls of a 4-block super-batch are forced (via
a data-dependency gate) to precede the Exp calls, so the ~2.7us
ACT_TABLE_LOAD set switches happen a handful of times instead of per
block. Square (used for the row stats) is in every table set.
"""

import os
import numpy as np

N_CORES = 8
C = 1024          # prototypes (classes)
D = 2048          # feature dim
Q = 8192          # total queries
QS = Q // N_CORES # queries per core
EPS = 1e-6
KT = D // 128     # contraction k-tiles
QB = QS // 128    # query blocks per core
CH = 512          # class half width
NH = C // CH      # number of class halves

# matmul compute dtype: "f32r" (full-rate fp32 path) or "f32" (4x slower)
MM_DTYPE = os.environ.get("KNN_MM_DTYPE", "f32r")

_BUILT = {}


def _build_program(mm_dtype: str):
    import concourse.mybir as mybir
    import concourse.tile as tile
    from concourse import bacc
    from contextlib import ExitStack

    f32 = mybir.dt.float32
    u8 = mybir.dt.uint8
    AL = mybir.AluOpType
    AF = mybir.ActivationFunctionType
    AX = mybir.AxisListType
    mmdt = mybir.dt.float32r if mm_dtype == "f32r" else mybir.dt.float32

    nc = bacc.Bacc("TRN2", target_bir_lowering=False, debug=False,
                   num_devices=N_CORES)

    ins = {}
    for name in ("qt_r", "qt_f"):
        ins[name] = nc.dram_tensor(name, [D, QS], mmdt, kind="ExternalInput").ap()
    for name in ("pt_r", "pt_f"):
        ins[name] = nc.dram_tensor(name, [D, C], mmdt, kind="ExternalInput").ap()

    outs = {}
    for name in ("p_r", "p_f"):
        outs[name] = nc.dram_tensor(name, [QS, C], f32, kind="ExternalOutput").ap()
    for name in ("c_r", "h_r", "c_f", "h_f"):
        outs[name] = nc.dram_tensor(name, [QS], f32, kind="ExternalOutput").ap()
    outs["sel"] = nc.dram_tensor("sel", [QS], u8, kind="ExternalOutput").ap()

    with tile.TileContext(nc) as tc, ExitStack() as ctx:
        const = ctx.enter_context(tc.tile_pool(name="const", bufs=1))
        # qt: both modalities can be resident (current + prefetch)
        qt_pool = ctx.enter_context(tc.tile_pool(name="qt", bufs=KT + 8))
        # pt: one 512-wide half (16 k-tiles) + lookahead
        pt_pool = ctx.enter_context(tc.tile_pool(name="pt", bufs=KT + 1))
        scr_pool = ctx.enter_context(tc.tile_pool(name="scr", bufs=1))
        scrh_pool = ctx.enter_context(tc.tile_pool(name="scrh", bufs=2))
        aug_pool = ctx.enter_context(tc.tile_pool(name="aug", bufs=1))
        spsum = ctx.enter_context(tc.tile_pool(name="spsum", bufs=3, space="PSUM"))
        mpsum = ctx.enter_context(tc.tile_pool(name="mpsum", bufs=4, space="PSUM"))
        dist_pool = ctx.enter_context(tc.tile_pool(name="dist", bufs=4))
        u0_pool = ctx.enter_context(tc.tile_pool(name="u0", bufs=QB))
        u1_pool = ctx.enter_context(tc.tile_pool(name="u1", bufs=2))
        post_pool = ctx.enter_context(tc.tile_pool(name="post", bufs=2))
        small = ctx.enter_context(tc.tile_pool(name="small", bufs=2))
        stat_pool = ctx.enter_context(tc.tile_pool(name="stat", bufs=1))

        ones_f = const.tile([128, 1], f32)
        nc.vector.memset(ones_f[:], 1.0)
        ones_row_f = const.tile([1, 1024], f32)
        nc.vector.memset(ones_row_f[:], 1.0)
        if mmdt == f32:
            ones, ones_row = ones_f, ones_row_f
        else:
            ones = const.tile([128, 1], mmdt)
            nc.scalar.copy(ones[:], ones_f[:])
            ones_row = const.tile([1, 1024], mmdt)
            nc.scalar.copy(ones_row[:], ones_row_f[:])

        # persistent per-modality stats
        stt = {}
        for m in ("r", "f"):
            for nm in ("S0", "t0", "m0", "S", "t", "mx", "conf", "ent"):
                stt[m, nm] = stat_pool.tile([128, QB], f32, tag=f"{nm}_{m}",
                                            name=f"{nm}_{m}")

        qts = {}
        qaugs = {}
        u0_keep = {}

        # ---------- query-side load + alpha stats ----------
        def load_q_side(m):
            qt_d = ins[f"qt_{m}"]
            qt = []
            for k in range(KT):
                tq = qt_pool.tile([128, QS], mmdt, tag="qt", name=f"qt_{m}_{k}")
                nc.sync.dma_start(tq[:], qt_d[k * 128:(k + 1) * 128, :])
                qt.append(tq)
            qts[m] = qt
            a_ps = [spsum.tile([1, 512], f32, tag="sp", name=f"aps_{m}_{i}")
                    for i in range(2)]
            for k in range(KT):
                sq = scr_pool.tile([128, QS], mmdt, tag="scr")
                nc.vector.scalar_tensor_tensor(
                    sq[:], qt[k][:], 2.0 * EPS, qt[k][:],
                    op0=AL.add, op1=AL.mult)
                for ci in range(2):
                    nc.tensor.matmul(
                        a_ps[ci][:], ones[:], sq[:, ci * 512:(ci + 1) * 512],
                        start=(k == 0), stop=(k == KT - 1))
            alpha_row = aug_pool.tile([1, QS], mmdt, tag="arow",
                                      name=f"arow_{m}")
            for ci in range(2):
                nc.scalar.mul(alpha_row[0:1, ci * 512:(ci + 1) * 512],
                              a_ps[ci][:], -0.5)
            qaug = aug_pool.tile([2, QS], mmdt, tag="qaug", name=f"qaug_{m}", bufs=2)
            nc.sync.dma_start(qaug[0:1, :], alpha_row[:])
            nc.sync.dma_start(qaug[1:2, :], ones_row[0:1, 0:QS])
            qaugs[m] = qaug

        # ---------- prototype-half load + beta stats ----------
        def load_p_half(m, h):
            pt_d = ins[f"pt_{m}"]
            csl = slice(h * CH, (h + 1) * CH)
            pt = []
            for k in range(KT):
                tp = pt_pool.tile([128, CH], mmdt, tag="pt",
                                  name=f"pt_{m}{h}_{k}")
                nc.sync.dma_start(tp[:], pt_d[k * 128:(k + 1) * 128, csl])
                pt.append(tp)
            b_ps = spsum.tile([1, CH], f32, tag="sp", name=f"bps_{m}{h}")
            for k in range(KT):
                sq = scrh_pool.tile([128, CH], mmdt, tag="scrh")
                nc.vector.scalar_tensor_tensor(
                    sq[:], pt[k][:], -2.0 * EPS, pt[k][:],
                    op0=AL.add, op1=AL.mult)
                nc.tensor.matmul(b_ps[:], ones[:], sq[:],
                                 start=(k == 0), stop=(k == KT - 1))
            beta_row = aug_pool.tile([1, CH], mmdt, tag="brow",
                                     name=f"brow_{m}{h}")
            nc.scalar.mul(beta_row[0:1, :], b_ps[:], -0.5)
            paug = aug_pool.tile([2, CH], mmdt, tag="paug", name=f"paug_{m}{h}", bufs=2)
            nc.sync.dma_start(paug[0:1, :], ones_row[0:1, 0:CH])
            nc.sync.dma_start(paug[1:2, :], beta_row[:])
            return pt, paug

        # ---------- one (modality, class-half) phase ----------
        def run_phase(m, h):
            qt = qts[m]
            pt, paug = load_p_half(m, h)
            qaug = qaugs[m]

            dists = []
            for qb in range(QB):
                qsl = slice(qb * 128, (qb + 1) * 128)
                ps = mpsum.tile([128, CH], f32, tag="ps", name=f"ps_{m}{h}_{qb}")
                for k in range(KT):
                    nc.tensor.matmul(ps[:], qt[k][:, qsl], pt[k][:],
                                     start=(k == 0), stop=False)
                nc.tensor.matmul(ps[:], qaug[:, qsl], paug[:],
                                 start=False, stop=True)
                dist = dist_pool.tile([128, CH], f32, tag="dist",
                                      name=f"dist_{m}{h}_{qb}")
                nc.scalar.activation(dist[:], ps[:], AF.Sqrt, bias=0.0,
                                     scale=-2.0)
                dists.append(dist)

            # table-set gates per 4-block super-batch
            SBW = 4
            neg1s = {}
            for sb in range((QB + SBW - 1) // SBW):
                last = min(sb * SBW + SBW, QB) - 1
                neg1 = small.tile([128, 1], f32, tag="neg1",
                                  name=f"neg1_{m}{h}_{sb}", bufs=4)
                nc.vector.tensor_scalar(neg1[:], dists[last][:, 0:1], 0.0,
                                        -1.0, op0=AL.mult, op1=AL.add)
                neg1s[sb] = neg1

            for qb in range(QB):
                dist = dists[qb]
                neg1 = neg1s[qb // SBW]
                qsl = slice(qb * 128, (qb + 1) * 128)
                col = slice(qb, qb + 1)
                if h == 0:
                    u = u0_pool.tile([128, CH], f32, tag="u0",
                                     name=f"u0_{m}_{qb}")
                    nc.scalar.activation(u[:], dist[:], AF.Exp, bias=0.0,
                                         scale=neg1[:],
                                         accum_out=stt[m, "S0"][:, col])
                    scr = scrh_pool.tile([128, CH], f32, tag="scrh")
                    nc.vector.scalar_tensor_tensor(
                        scr[:], u[:], 1.0, dist[:], op0=AL.mult, op1=AL.mult,
                        accum_out=stt[m, "t0"][:, col])
                    nc.vector.reduce_max(stt[m, "m0"][:, col], u[:], axis=AX.X)
                    u0_keep[(m, qb)] = u
                else:
                    u0 = u0_keep.pop((m, qb))
                    u = u1_pool.tile([128, CH], f32, tag="u1")
                    S1 = small.tile([128, 1], f32, tag="S1")
                    nc.scalar.activation(u[:], dist[:], AF.Exp, bias=0.0,
                                         scale=neg1[:], accum_out=S1[:])
                    t1 = small.tile([128, 1], f32, tag="t1")
                    scr = scrh_pool.tile([128, CH], f32, tag="scrh")
                    nc.vector.scalar_tensor_tensor(
                        scr[:], u[:], 1.0, dist[:], op0=AL.mult, op1=AL.mult,
                        accum_out=t1[:])
                    m1 = small.tile([128, 1], f32, tag="m1")
                    nc.vector.reduce_max(m1[:], u[:], axis=AX.X)

                    nc.vector.tensor_add(stt[m, "S"][:, col],
                                         stt[m, "S0"][:, col], S1[:])
                    nc.vector.tensor_add(stt[m, "t"][:, col],
                                         stt[m, "t0"][:, col], t1[:])
                    nc.vector.tensor_max(stt[m, "mx"][:, col],
                                         stt[m, "m0"][:, col], m1[:])
                    rS = small.tile([128, 1], f32, tag="rS")
                    nc.vector.reciprocal(rS[:], stt[m, "S"][:, col])
                    pw = post_pool.tile([128, C], f32, tag="post")
                    nc.vector.tensor_scalar_mul(pw[:, 0:CH], u0[:], rS[:])
                    nc.vector.tensor_scalar_mul(pw[:, CH:C], u[:], rS[:])
                    nc.gpsimd.dma_start(outs[f"p_{m}"][qsl, :], pw[:])

            if h == NH - 1:
                rS8 = small.tile([128, QB], f32, tag=f"rS8_{m}",
                                 name=f"rS8_{m}")
                nc.vector.reciprocal(rS8[:], stt[m, "S"][:])
                nc.vector.tensor_mul(stt[m, "conf"][:], stt[m, "mx"][:], rS8[:])
                lnS8 = small.tile([128, QB], f32, tag=f"lnS8_{m}",
                                  name=f"lnS8_{m}")
                nc.scalar.activation(lnS8[:], stt[m, "S"][:], AF.Ln)
                trs = small.tile([128, QB], f32, tag=f"trs_{m}",
                                 name=f"trs_{m}")
                nc.vector.tensor_mul(trs[:], stt[m, "t"][:], rS8[:])
                nc.vector.tensor_add(stt[m, "ent"][:], trs[:], lnS8[:])

        # ---------- phase schedule ----------
        load_q_side("r")
        run_phase("r", 0)
        load_q_side("f")      # prefetch overlaps (r,1) GEMM
        run_phase("r", 1)
        run_phase("f", 0)
        run_phase("f", 1)

        # ---------- select mask + stat outputs ----------
        def col_view(ap1d):
            return ap1d.rearrange("(a b) -> b a", a=QB)

        gh = small.tile([128, QB], f32, tag="gh")
        nc.vector.tensor_tensor(gh[:], stt["r", "ent"][:], stt["f", "ent"][:],
                                op=AL.is_gt)
        gc = small.tile([128, QB], f32, tag="gc")
        nc.vector.tensor_tensor(gc[:], stt["r", "conf"][:], stt["f", "conf"][:],
                                op=AL.is_gt)
        selm = small.tile([128, QB], f32, tag="selm")
        nc.vector.tensor_tensor(selm[:], gh[:], gc[:], op=AL.logical_and)
        sel8 = small.tile([128, QB], u8, tag="sel8")
        nc.vector.tensor_copy(sel8[:], selm[:])
        nc.gpsimd.dma_start(col_view(outs["sel"]), sel8[:])
        for m in ("r", "f"):
            nc.gpsimd.dma_start(col_view(outs[f"c_{m}"]), stt[m, "conf"][:])
            nc.gpsimd.dma_start(col_view(outs[f"h_{m}"]), stt[m, "ent"][:])

    nc.compile()
    return nc


def _get_program(mm_dtype: str = MM_DTYPE):
    if mm_dtype not in _BUILT:
        _BUILT[mm_dtype] = _build_program(mm_dtype)
    return _BUILT[mm_dtype]


def _make_in_maps(context_rgb_features, context_flow_features,
                  target_rgb_features, target_flow_features):
    pt_r = np.ascontiguousarray(np.asarray(context_rgb_features).T)
    pt_f = np.ascontiguousarray(np.asarray(context_flow_features).T)
    qt_r = np.asarray(target_rgb_features).T   # [D, Q]
    qt_f = np.asarray(target_flow_features).T

    in_maps = []
    for i in range(N_CORES):
        qsl = slice(i * QS, (i + 1) * QS)
        in_maps.append({
            "qt_r": np.ascontiguousarray(qt_r[:, qsl]),
            "qt_f": np.ascontiguousarray(qt_f[:, qsl]),
            "pt_r": pt_r,
            "pt_f": pt_f,
        })
    return in_maps


def kernel(context_rgb_features, context_flow_features,
           target_rgb_features, target_flow_features):
    from concourse.bass_utils import run_bass_kernel_spmd

    nc = _get_program()
    in_maps = _make_in_maps(context_rgb_features, context_flow_features,
                            target_rgb_features, target_flow_features)
    res = run_bass_kernel_spmd(nc, in_maps, core_ids=list(range(N_CORES)))
    r = res.results

    def cat(name):
        return np.concatenate([r[i][name] for i in range(N_CORES)], axis=0)

    p_r, c_r, h_r = cat("p_r"), cat("c_r"), cat("h_r")
    p_f, c_f, h_f = cat("p_f"), cat("c_f"), cat("h_f")
    sel = cat("sel").astype(bool)
    return p_r, c_r, h_r, p_f, c_f, h_f, sel


# revision 24
# speedup vs baseline: 1.1643x; 1.1643x over previous
"""Trainium2 Bass kernel for dual-modality soft-kNN posterior (retrieval_knn).

Computes, for rgb and flow feature sets:
    dist[q,c] = ||q - p + eps||_2          (torch PairwiseDistance eps)
    post      = exp(-dist) / row_sum
    conf      = row max of post
    ent       = row entropy of post
and select mask (ent_r > ent_f) & (conf_r > conf_f).

Strategy: data-parallel over queries across 8 NeuronCores; prototypes
replicated. Host passes transposed ([D, *]) copies of all feature
matrices so the TensorEngine can contract over D directly.

Math on device (per modality):
    alpha[q] = sum_d q_d^2 + 2*eps*q_d
    beta[c]  = sum_d p_d^2 - 2*eps*p_d      (+ D*eps^2, dropped: < f32 ulp)
    g[q,c]   = sum_d q_d p_d - alpha[q]/2 - beta[c]/2   (K=2 augmented row)
    dist     = sqrt(-2*g)
    u = exp(-dist); S = sum_c u;  post = u/S
    conf = max_c(u)/S;  ent = ln(S) + (sum_c u*dist)/S

Pipelining: classes are processed in two 512-wide halves per modality
(4 phases total). Only one modality's query set plus one prototype
half needs to be resident at a time, which leaves room to prefetch the
next phase's tiles, keeping the TensorEngine dense (and out of the HAM
cold-clock state) across phase boundaries.

ACT table sets: all Sqrt calls of a 4-block super-batch are forced (via
a data-dependency gate) to precede the Exp calls, so the ~2.7us
ACT_TABLE_LOAD set switches happen a handful of times instead of per
block. Square (used for the row stats) is in every table set.
"""

import os
import numpy as np

N_CORES = 8
C = 1024          # prototypes (classes)
D = 2048          # feature dim
Q = 8192          # total queries
QS = Q // N_CORES # queries per core
EPS = 1e-6
KT = D // 128     # contraction k-tiles
QB = QS // 128    # query blocks per core
CH = 512          # class half width
NH = C // CH      # number of class halves

# matmul compute dtype: "f32r" (full-rate fp32 path) or "f32" (4x slower)
MM_DTYPE = os.environ.get("KNN_MM_DTYPE", "f32r")

_BUILT = {}


def _build_program(mm_dtype: str):
    import concourse.mybir as mybir
    import concourse.tile as tile
    from concourse import bacc
    from contextlib import ExitStack

    f32 = mybir.dt.float32
    u8 = mybir.dt.uint8
    AL = mybir.AluOpType
    AF = mybir.ActivationFunctionType
    AX = mybir.AxisListType
    mmdt = mybir.dt.float32r if mm_dtype == "f32r" else mybir.dt.float32

    nc = bacc.Bacc("TRN2", target_bir_lowering=False, debug=False,
                   num_devices=N_CORES)

    ins = {}
    for name in ("qt_r", "qt_f"):
        ins[name] = nc.dram_tensor(name, [D, QS], mmdt, kind="ExternalInput").ap()
    for name in ("pt_r", "pt_f"):
        ins[name] = nc.dram_tensor(name, [D, C], mmdt, kind="ExternalInput").ap()

    outs = {}
    for name in ("p_r", "p_f"):
        outs[name] = nc.dram_tensor(name, [QS, C], f32, kind="ExternalOutput").ap()
    for name in ("c_r", "h_r", "c_f", "h_f"):
        outs[name] = nc.dram_tensor(name, [QS], f32, kind="ExternalOutput").ap()
    outs["sel"] = nc.dram_tensor("sel", [QS], u8, kind="ExternalOutput").ap()

    with tile.TileContext(nc) as tc, ExitStack() as ctx:
        const = ctx.enter_context(tc.tile_pool(name="const", bufs=1))
        # qt: both modalities can be resident (current + prefetch)
        qt_pool = ctx.enter_context(tc.tile_pool(name="qt", bufs=KT + 7))
        # pt: one 512-wide half (16 k-tiles) + lookahead
        pt_pool = ctx.enter_context(tc.tile_pool(name="pt", bufs=KT + 2))
        scr_pool = ctx.enter_context(tc.tile_pool(name="scr", bufs=1))
        scrh_pool = ctx.enter_context(tc.tile_pool(name="scrh", bufs=2))
        aug_pool = ctx.enter_context(tc.tile_pool(name="aug", bufs=1))
        spsum = ctx.enter_context(tc.tile_pool(name="spsum", bufs=3, space="PSUM"))
        mpsum = ctx.enter_context(tc.tile_pool(name="mpsum", bufs=5, space="PSUM"))
        dist_pool = ctx.enter_context(tc.tile_pool(name="dist", bufs=4))
        u0_pool = ctx.enter_context(tc.tile_pool(name="u0", bufs=QB))
        u1_pool = ctx.enter_context(tc.tile_pool(name="u1", bufs=2))
        post_pool = ctx.enter_context(tc.tile_pool(name="post", bufs=2))
        small = ctx.enter_context(tc.tile_pool(name="small", bufs=2))
        stat_pool = ctx.enter_context(tc.tile_pool(name="stat", bufs=1))

        ones_f = const.tile([128, 1], f32)
        nc.vector.memset(ones_f[:], 1.0)
        ones_row_f = const.tile([1, 1024], f32)
        nc.vector.memset(ones_row_f[:], 1.0)
        if mmdt == f32:
            ones, ones_row = ones_f, ones_row_f
        else:
            ones = const.tile([128, 1], mmdt)
            nc.scalar.copy(ones[:], ones_f[:])
            ones_row = const.tile([1, 1024], mmdt)
            nc.scalar.copy(ones_row[:], ones_row_f[:])

        # persistent per-modality stats
        stt = {}
        for m in ("r", "f"):
            for nm in ("S0", "t0", "m0", "S", "t", "mx", "conf", "ent"):
                stt[m, nm] = stat_pool.tile([128, QB], f32, tag=f"{nm}_{m}",
                                            name=f"{nm}_{m}")

        qts = {}
        qaugs = {}
        u0_keep = {}

        # ---------- query-side load + alpha stats ----------
        def load_q_side(m):
            qt_d = ins[f"qt_{m}"]
            qt = []
            for k in range(KT):
                tq = qt_pool.tile([128, QS], mmdt, tag="qt", name=f"qt_{m}_{k}")
                nc.sync.dma_start(tq[:], qt_d[k * 128:(k + 1) * 128, :])
                qt.append(tq)
            qts[m] = qt
            a_ps = [spsum.tile([1, 512], f32, tag="sp", name=f"aps_{m}_{i}")
                    for i in range(2)]
            for k in range(KT):
                for ci in range(2):
                    qsl = slice(ci * 512, (ci + 1) * 512)
                    sq = scr_pool.tile([128, 512], mmdt, tag="scr")
                    nc.vector.scalar_tensor_tensor(
                        sq[:], qt[k][:, qsl], 2.0 * EPS, qt[k][:, qsl],
                        op0=AL.add, op1=AL.mult)
                    nc.tensor.matmul(
                        a_ps[ci][:], ones[:], sq[:],
                        start=(k == 0), stop=(k == KT - 1))
            alpha_row = aug_pool.tile([1, QS], mmdt, tag="arow",
                                      name=f"arow_{m}")
            for ci in range(2):
                nc.scalar.mul(alpha_row[0:1, ci * 512:(ci + 1) * 512],
                              a_ps[ci][:], -0.5)
            qaug = aug_pool.tile([2, QS], mmdt, tag="qaug", name=f"qaug_{m}", bufs=2)
            nc.sync.dma_start(qaug[0:1, :], alpha_row[:])
            nc.sync.dma_start(qaug[1:2, :], ones_row[0:1, 0:QS])
            qaugs[m] = qaug

        # ---------- prototype-half load + beta stats ----------
        def load_p_half(m, h):
            pt_d = ins[f"pt_{m}"]
            csl = slice(h * CH, (h + 1) * CH)
            pt = []
            for k in range(KT):
                tp = pt_pool.tile([128, CH], mmdt, tag="pt",
                                  name=f"pt_{m}{h}_{k}")
                nc.scalar.dma_start(tp[:], pt_d[k * 128:(k + 1) * 128, csl])
                pt.append(tp)
            b_ps = spsum.tile([1, CH], f32, tag="sp", name=f"bps_{m}{h}")
            for k in range(KT):
                sq = scrh_pool.tile([128, CH], mmdt, tag="scrh")
                nc.vector.scalar_tensor_tensor(
                    sq[:], pt[k][:], -2.0 * EPS, pt[k][:],
                    op0=AL.add, op1=AL.mult)
                nc.tensor.matmul(b_ps[:], ones[:], sq[:],
                                 start=(k == 0), stop=(k == KT - 1))
            beta_row = aug_pool.tile([1, CH], mmdt, tag="brow",
                                     name=f"brow_{m}{h}")
            nc.scalar.mul(beta_row[0:1, :], b_ps[:], -0.5)
            paug = aug_pool.tile([2, CH], mmdt, tag="paug", name=f"paug_{m}{h}", bufs=2)
            nc.sync.dma_start(paug[0:1, :], ones_row[0:1, 0:CH])
            nc.sync.dma_start(paug[1:2, :], beta_row[:])
            return pt, paug

        # ---------- one (modality, class-half) phase ----------
        def run_phase(m, h):
            qt = qts[m]
            pt, paug = load_p_half(m, h)
            qaug = qaugs[m]

            dists = []
            for qb in range(QB):
                qsl = slice(qb * 128, (qb + 1) * 128)
                ps = mpsum.tile([128, CH], f32, tag="ps", name=f"ps_{m}{h}_{qb}")
                for k in range(KT):
                    nc.tensor.matmul(ps[:], qt[k][:, qsl], pt[k][:],
                                     start=(k == 0), stop=False)
                nc.tensor.matmul(ps[:], qaug[:, qsl], paug[:],
                                 start=False, stop=True)
                dist = dist_pool.tile([128, CH], f32, tag="dist",
                                      name=f"dist_{m}{h}_{qb}")
                nc.scalar.activation(dist[:], ps[:], AF.Sqrt, bias=0.0,
                                     scale=-2.0)
                dists.append(dist)

            # table-set gates per 4-block super-batch
            SBW = 2 if (m, h) == ("f", 1) else 4
            neg1s = {}
            for sb in range((QB + SBW - 1) // SBW):
                last = min(sb * SBW + SBW, QB) - 1
                neg1 = small.tile([128, 1], f32, tag="neg1",
                                  name=f"neg1_{m}{h}_{sb}", bufs=4)
                nc.vector.tensor_scalar(neg1[:], dists[last][:, 0:1], 0.0,
                                        -1.0, op0=AL.mult, op1=AL.add)
                neg1s[sb] = neg1

            for qb in range(QB):
                dist = dists[qb]
                neg1 = neg1s[qb // SBW]
                qsl = slice(qb * 128, (qb + 1) * 128)
                col = slice(qb, qb + 1)
                if h == 0:
                    u = u0_pool.tile([128, CH], f32, tag="u0",
                                     name=f"u0_{m}_{qb}")
                    nc.scalar.activation(u[:], dist[:], AF.Exp, bias=0.0,
                                         scale=neg1[:],
                                         accum_out=stt[m, "S0"][:, col])
                    scr = scrh_pool.tile([128, CH], f32, tag="scrh")
                    nc.vector.scalar_tensor_tensor(
                        scr[:], u[:], 1.0, dist[:], op0=AL.mult, op1=AL.mult,
                        accum_out=stt[m, "t0"][:, col])
                    nc.vector.reduce_max(stt[m, "m0"][:, col], u[:], axis=AX.X)
                    u0_keep[(m, qb)] = u
                else:
                    u0 = u0_keep.pop((m, qb))
                    u = u1_pool.tile([128, CH], f32, tag="u1")
                    S1 = small.tile([128, 1], f32, tag="S1")
                    nc.scalar.activation(u[:], dist[:], AF.Exp, bias=0.0,
                                         scale=neg1[:], accum_out=S1[:])
                    t1 = small.tile([128, 1], f32, tag="t1")
                    scr = scrh_pool.tile([128, CH], f32, tag="scrh")
                    nc.vector.scalar_tensor_tensor(
                        scr[:], u[:], 1.0, dist[:], op0=AL.mult, op1=AL.mult,
                        accum_out=t1[:])
                    m1 = small.tile([128, 1], f32, tag="m1")
                    nc.vector.reduce_max(m1[:], u[:], axis=AX.X)

                    nc.vector.tensor_add(stt[m, "S"][:, col],
                                         stt[m, "S0"][:, col], S1[:])
                    nc.vector.tensor_add(stt[m, "t"][:, col],
                                         stt[m, "t0"][:, col], t1[:])
                    nc.vector.tensor_max(stt[m, "mx"][:, col],
                                         stt[m, "m0"][:, col], m1[:])
                    rS = small.tile([128, 1], f32, tag="rS")
                    nc.vector.reciprocal(rS[:], stt[m, "S"][:, col])
                    pw = post_pool.tile([128, C], f32, tag="post")
                    nc.vector.tensor_scalar_mul(pw[:, 0:CH], u0[:], rS[:])
                    nc.vector.tensor_scalar_mul(pw[:, CH:C], u[:], rS[:])
                    if qb % 2 == 0:
                        nc.gpsimd.dma_start(outs[f"p_{m}"][qsl, :], pw[:])
                    else:
                        nc.scalar.dma_start(outs[f"p_{m}"][qsl, :], pw[:])

            if h == NH - 1:
                rS8 = small.tile([128, QB], f32, tag=f"rS8_{m}",
                                 name=f"rS8_{m}")
                nc.vector.reciprocal(rS8[:], stt[m, "S"][:])
                nc.vector.tensor_mul(stt[m, "conf"][:], stt[m, "mx"][:], rS8[:])
                lnS8 = small.tile([128, QB], f32, tag=f"lnS8_{m}",
                                  name=f"lnS8_{m}")
                nc.scalar.activation(lnS8[:], stt[m, "S"][:], AF.Ln)
                trs = small.tile([128, QB], f32, tag=f"trs_{m}",
                                 name=f"trs_{m}")
                nc.vector.tensor_mul(trs[:], stt[m, "t"][:], rS8[:])
                nc.vector.tensor_add(stt[m, "ent"][:], trs[:], lnS8[:])

        # ---------- phase schedule ----------
        load_q_side("r")
        run_phase("r", 0)
        load_q_side("f")      # prefetch overlaps (r,1) GEMM
        run_phase("r", 1)
        run_phase("f", 0)
        run_phase("f", 1)

        # ---------- select mask + stat outputs ----------
        def col_view(ap1d):
            return ap1d.rearrange("(a b) -> b a", a=QB)

        gh = small.tile([128, QB], f32, tag="gh")
        nc.vector.tensor_tensor(gh[:], stt["r", "ent"][:], stt["f", "ent"][:],
                                op=AL.is_gt)
        gc = small.tile([128, QB], f32, tag="gc")
        nc.vector.tensor_tensor(gc[:], stt["r", "conf"][:], stt["f", "conf"][:],
                                op=AL.is_gt)
        selm = small.tile([128, QB], f32, tag="selm")
        nc.vector.tensor_tensor(selm[:], gh[:], gc[:], op=AL.logical_and)
        sel8 = small.tile([128, QB], u8, tag="sel8")
        nc.vector.tensor_copy(sel8[:], selm[:])
        nc.gpsimd.dma_start(col_view(outs["sel"]), sel8[:])
        for m in ("r", "f"):
            nc.gpsimd.dma_start(col_view(outs[f"c_{m}"]), stt[m, "conf"][:])
            nc.gpsimd.dma_start(col_view(outs[f"h_{m}"]), stt[m, "ent"][:])

    nc.compile()
    return nc


def _get_program(mm_dtype: str = MM_DTYPE):
    if mm_dtype not in _BUILT:
        _BUILT[mm_dtype] = _build_program(mm_dtype)
    return _BUILT[mm_dtype]


def _make_in_maps(context_rgb_features, context_flow_features,
                  target_rgb_features, target_flow_features):
    pt_r = np.ascontiguousarray(np.asarray(context_rgb_features).T)
    pt_f = np.ascontiguousarray(np.asarray(context_flow_features).T)
    qt_r = np.asarray(target_rgb_features).T   # [D, Q]
    qt_f = np.asarray(target_flow_features).T

    in_maps = []
    for i in range(N_CORES):
        qsl = slice(i * QS, (i + 1) * QS)
        in_maps.append({
            "qt_r": np.ascontiguousarray(qt_r[:, qsl]),
            "qt_f": np.ascontiguousarray(qt_f[:, qsl]),
            "pt_r": pt_r,
            "pt_f": pt_f,
        })
    return in_maps


def kernel(context_rgb_features, context_flow_features,
           target_rgb_features, target_flow_features):
    from concourse.bass_utils import run_bass_kernel_spmd

    nc = _get_program()
    in_maps = _make_in_maps(context_rgb_features, context_flow_features,
                            target_rgb_features, target_flow_features)
    res = run_bass_kernel_spmd(nc, in_maps, core_ids=list(range(N_CORES)))
    r = res.results

    def cat(name):
        return np.concatenate([r[i][name] for i in range(N_CORES)], axis=0)

    p_r, c_r, h_r = cat("p_r"), cat("c_r"), cat("h_r")
    p_f, c_f, h_f = cat("p_f"), cat("c_f"), cat("h_f")
    sel = cat("sel").astype(bool)
    return p_r, c_r, h_r, p_f, c_f, h_f, sel
